# revision 2
# baseline (speedup 1.0000x reference)
"""MDN-RNN mixture-density loss kernel for Trainium2, SPMD over 8 NeuronCores.

Math (per token row i):
    means/logstds: [K, D] slices of s_mean/s_logstd rows
    z      = (target - mean_k) * exp(-logstd_k)
    logp_k = -0.5 * sum_d z^2 - sum_d logstd_k
    loss   = -mean_i logsumexp_k(log_mix_coeffs + logp_k)

Sharding: data-parallel on the token dim N=16384 -> 2048 rows per core,
no cross-device communication; each core emits a [128,1] partial sum of
per-row -logsumexp values, combined into the scalar mean on the host.

Precision/traffic: target+mean are shipped as bf16 and logstd as
fp8-e4m3 (rel err on the final loss ~8e-4, well inside the 2e-2 gate).
fp8 on logstd is free compute-wise: the only consumers are the ACT exp
(dtype-flat rate) and reductions (no DVE fast mode either way), while
it cuts that tensor's HBM bytes 4x.

Engine split per 128-row tile (bf16 data, all [P, K, D] = [128, 5, 1088]):
    ACT:  e1 = exp(-logstd) (one 3D pass), z^2 sum for ACT_SQ_K k-slices
          (Square w/ accumulate), per-tile logsumexp exp w/ accumulate
    DVE:  z = diff*e1 (3D tt, 2x bf16 mode), z^2 sum for the remaining
          k-slices (scalar_tensor_tensor w/ accum_out), sls = sum(logstd)
          (one grouped 3D reduce), logsumexp max
    Pool: diff = target(bcast over k) - mean (gpsimd tensor_tensor),
          logsumexp score+mix add
Ln is deferred to a single [128,T] pass after the loop so the ACT table
set {Exp, Square, Copy} never swaps inside the loop.
"""

import sys

if "/opt/trn_rl_repo" not in sys.path:
    sys.path.insert(0, "/opt/trn_rl_repo")

import numpy as np
import ml_dtypes

N = 16384
K = 5
D = 1088
KD = K * D
NCORES = 8
R = N // NCORES          # 2048 rows per core
P = 128                  # partitions
T = R // P               # 16 tiles per core

ACT_SQ_K = 4             # k-slices of sum(z^2) on ACT; rest via DVE stt

_NC = None


def _build():
    import concourse.bacc as bacc
    import concourse.bass as bass
    import concourse.tile as tile
    from concourse import mybir

    AF = mybir.ActivationFunctionType
    AL = mybir.AluOpType
    AX = mybir.AxisListType
    f32 = mybir.dt.float32
    bf16 = mybir.dt.bfloat16
    f8 = mybir.dt.float8e4

    nc = bacc.Bacc("TRN2", debug=False)
    tgt = nc.dram_tensor("tgt", [R, D], bf16, kind="ExternalInput").ap()
    mean = nc.dram_tensor("mean", [R, KD], bf16, kind="ExternalInput").ap()
    lstd = nc.dram_tensor("lstd", [R, KD], f8, kind="ExternalInput").ap()
    lmx = nc.dram_tensor("lmx", [P, T * K], f32, kind="ExternalInput").ap()
    out = nc.dram_tensor("partial", [P, 1], f32, kind="ExternalOutput").ap()

    with tile.TileContext(nc) as tc:
        with (
            tc.tile_pool(name="tgt_p", bufs=3) as tgt_p,
            tc.tile_pool(name="mean_p", bufs=3) as mean_p,
            tc.tile_pool(name="lstd_p", bufs=3) as lstd_p,
            tc.tile_pool(name="e1_p", bufs=2) as e1_p,
            tc.tile_pool(name="diff_p", bufs=2) as diff_p,
            tc.tile_pool(name="small_p", bufs=3) as small_p,
            tc.tile_pool(name="persist", bufs=1) as persist,
        ):
            t_lmx = persist.tile([P, T * K], f32)
            nc.sync.dma_start(out=t_lmx, in_=lmx)
            t_nm = persist.tile([P, T], f32)      # per-tile -max_k score
            t_sacc = persist.tile([P, T], f32)    # per-tile sum_k exp(score+nm)

            state = {}

            def emit_a(t):
                """Front: DMAs, sls reduce, e1 = exp(-logstd), diff on Pool."""
                rows = slice(t * P, (t + 1) * P)
                t_tgt = tgt_p.tile([P, D], bf16)
                t_mean = mean_p.tile([P, K, D], bf16)
                t_lstd = lstd_p.tile([P, K, D], f8)
                mean3 = mean[rows].rearrange("p (k d) -> p k d", k=K)
                lstd3 = lstd[rows].rearrange("p (k d) -> p k d", k=K)
                if t != 0:
                    nc.sync.dma_start(out=t_lstd, in_=lstd3)
                    nc.sync.dma_start(out=t_tgt, in_=tgt[rows])
                    nc.sync.dma_start(out=t_mean, in_=mean3)
                else:
                    # chunked so first compute starts after ~1/5 of the load
                    nc.sync.dma_start(out=t_lstd[:, 0, :], in_=lstd3[:, 0, :])
                    nc.sync.dma_start(out=t_tgt, in_=tgt[rows])
                    for k in range(1, K):
                        nc.sync.dma_start(out=t_lstd[:, k, :], in_=lstd3[:, k, :])
                    for k in range(K):
                        nc.sync.dma_start(out=t_mean[:, k, :], in_=mean3[:, k, :])

                # sls_k = sum_d logstd: one grouped 3D reduce on DVE
                t_sls = small_p.tile([P, K], f32)
                nc.vector.tensor_reduce(
                    out=t_sls, in_=t_lstd, axis=AX.X, op=AL.add
                )
                # e1 = exp(-logstd) on ACT (fp8 in, bf16 out)
                t_e1 = e1_p.tile([P, K, D], bf16)
                nc.scalar.activation(out=t_e1, in_=t_lstd, func=AF.Exp, scale=-1.0)

                # diff = target (broadcast over k) - mean on Pool
                t_diff = diff_p.tile([P, K, D], bf16)
                tgt_b = bass.AP(
                    tensor=t_tgt.tensor, offset=t_tgt.offset,
                    ap=[t_tgt.ap[0], [0, K], t_tgt.ap[1]],
                )
                nc.gpsimd.tensor_tensor(
                    out=t_diff, in0=tgt_b, in1=t_mean, op=AL.subtract
                )
                state[t] = (t_diff, t_e1, t_sls)

            def emit_b(t):
                """Back: z, squares w/ accumulate, logsumexp smalls."""
                t_diff, t_e1, t_sls = state.pop(t)
                # z = diff * e1 in place (3D DVE mult, 2x bf16 mode)
                nc.vector.tensor_tensor(out=t_diff, in0=t_diff, in1=t_e1, op=AL.mult)
                t_h = small_p.tile([P, K], f32)
                for k in range(ACT_SQ_K):
                    nc.scalar.activation(
                        out=t_diff[:, k, :], in_=t_diff[:, k, :], func=AF.Square,
                        accum_out=t_h[:, k : k + 1],
                    )
                for k in range(ACT_SQ_K, K):
                    nc.vector.scalar_tensor_tensor(
                        out=t_diff[:, k, :], in0=t_diff[:, k, :], scalar=1.0,
                        in1=t_diff[:, k, :], op0=AL.mult, op1=AL.mult,
                        accum_out=t_h[:, k : k + 1],
                    )

                # score_k = -0.5*h_k - sls_k + lmx_k ; nm = -max_k score
                t_q = small_p.tile([P, K], f32)
                nc.vector.scalar_tensor_tensor(
                    out=t_q, in0=t_h, scalar=-0.5, in1=t_sls,
                    op0=AL.mult, op1=AL.subtract,
                )
                t_c = small_p.tile([P, K], f32)
                nc.gpsimd.tensor_tensor(
                    out=t_c, in0=t_q, in1=t_lmx[:, t * K : (t + 1) * K], op=AL.add
                )
                nc.vector.tensor_reduce(
                    out=t_nm[:, t : t + 1], in_=t_c, axis=AX.X, op=AL.max, negate=True
                )
                # S_t = sum_k exp(score + nm)
                t_e = small_p.tile([P, K], f32)
                nc.scalar.activation(
                    out=t_e, in_=t_c, func=AF.Exp, bias=t_nm[:, t : t + 1],
                    scale=1.0, accum_out=t_sacc[:, t : t + 1],
                )

            # software-pipelined emission: tile t+1's front stage is queued
            # before tile t's back stage
            emit_a(0)
            for t in range(T):
                if t + 1 < T:
                    emit_a(t + 1)
                emit_b(t)

            # loss rows: nm_t - ln(S_t); single Ln pass keeps Ln out of the loop
            t_lns = persist.tile([P, T], f32)
            nc.scalar.activation(out=t_lns, in_=t_sacc, func=AF.Ln)
            t_accv = persist.tile([P, T], f32)
            nc.vector.tensor_tensor(out=t_accv, in0=t_nm, in1=t_lns, op=AL.subtract)
            t_tot = persist.tile([P, 1], f32)
            nc.vector.tensor_reduce(out=t_tot, in_=t_accv, axis=AX.X, op=AL.add)
            nc.sync.dma_start(out=out, in_=t_tot)

    nc.compile()
    return nc


def get_nc():
    global _NC
    if _NC is None:
        _NC = _build()
    return _NC


def make_in_maps(target, s_mean, s_logstd, log_mix_coeffs):
    target = np.asarray(target, dtype=np.float32).astype(ml_dtypes.bfloat16)
    s_mean = np.asarray(s_mean, dtype=np.float32).astype(ml_dtypes.bfloat16)
    s_logstd = np.asarray(s_logstd, dtype=np.float32).astype(ml_dtypes.float8_e4m3)
    lm = np.ascontiguousarray(np.asarray(log_mix_coeffs, dtype=np.float32))
    in_maps = []
    for c in range(NCORES):
        rows = slice(c * R, (c + 1) * R)
        # pack log-mix so tile t's [128, K] block sits at columns [t*K, (t+1)*K)
        lmx = lm[rows].reshape(T, P, K).transpose(1, 0, 2).reshape(P, T * K)
        in_maps.append({
            "tgt": np.ascontiguousarray(target[rows]),
            "mean": np.ascontiguousarray(s_mean[rows]),
            "lstd": np.ascontiguousarray(s_logstd[rows]),
            "lmx": np.ascontiguousarray(lmx),
        })
    return in_maps


def combine(results):
    total = sum(float(np.asarray(r["partial"], dtype=np.float64).sum()) for r in results)
    return np.float32(total / N)


def kernel(target, s_mean, s_logstd, log_mix_coeffs):
    from concourse.bass_utils import run_bass_kernel_spmd

    nc = get_nc()
    in_maps = make_in_maps(target, s_mean, s_logstd, log_mix_coeffs)
    res = run_bass_kernel_spmd(nc, in_maps, core_ids=list(range(NCORES)))
    return combine(res.results)


# revision 5
# speedup vs baseline: 1.5840x; 1.5840x over previous
"""MDN-RNN mixture-density loss kernel for Trainium2, SPMD over 8 NeuronCores.

Math (per token row i):
    means/logstds: [K, D] slices of s_mean/s_logstd rows
    z      = (target - mean_k) * exp(-logstd_k)
    logp_k = -0.5 * sum_d z^2 - sum_d logstd_k
    loss   = -mean_i logsumexp_k(log_mix_coeffs + logp_k)

Sharding: data-parallel on the token dim N=16384 -> 2048 rows per core,
no cross-device communication; each core emits a [128,1] partial sum of
per-row -logsumexp values, combined into the scalar mean on the host.

Precision/traffic: target+mean are shipped as bf16 and logstd as
fp8-e4m3 (rel err on the final loss ~8e-4, well inside the 2e-2 gate).
fp8 on logstd is free compute-wise: the only consumers are the ACT exp
(dtype-flat rate) and reductions (no DVE fast mode either way), while
it cuts that tensor's HBM bytes 4x.

Engine split per 128-row tile (bf16 data, all [P, K, D] = [128, 5, 1088]):
    ACT:  e1 = exp(-logstd) (one 3D pass), z^2 sum for ACT_SQ_K k-slices
          (Square w/ accumulate), per-tile logsumexp exp w/ accumulate
    DVE:  z = diff*e1 (3D tt, 2x bf16 mode), z^2 sum for the remaining
          k-slices (scalar_tensor_tensor w/ accum_out), sls = sum(logstd)
          (one grouped 3D reduce), logsumexp max
    Pool: diff = target(bcast over k) - mean (gpsimd tensor_tensor),
          logsumexp score+mix add
Ln is deferred to a single [128,T] pass after the loop so the ACT table
set {Exp, Square, Copy} never swaps inside the loop.
"""

import sys

if "/opt/trn_rl_repo" not in sys.path:
    sys.path.insert(0, "/opt/trn_rl_repo")

import numpy as np
import ml_dtypes

N = 16384
K = 5
D = 1088
KD = K * D
NCORES = 8
R = N // NCORES          # 2048 rows per core
P = 128                  # partitions
T = R // P               # 16 tiles per core

ACT_SQ_K = 5             # k-slices of sum(z^2) on ACT; rest via DVE stt

_NC = None


def _build():
    import concourse.bacc as bacc
    import concourse.bass as bass
    import concourse.tile as tile
    from concourse import mybir

    AF = mybir.ActivationFunctionType
    AL = mybir.AluOpType
    AX = mybir.AxisListType
    f32 = mybir.dt.float32
    bf16 = mybir.dt.bfloat16
    f8 = mybir.dt.float8e4

    nc = bacc.Bacc("TRN2", debug=False)
    tgt = nc.dram_tensor("tgt", [R, D], bf16, kind="ExternalInput").ap()
    mean = nc.dram_tensor("mean", [R, KD], bf16, kind="ExternalInput").ap()
    lstd = nc.dram_tensor("lstd", [R, KD], f8, kind="ExternalInput").ap()
    lmx = nc.dram_tensor("lmx", [P, T * K], f32, kind="ExternalInput").ap()
    out = nc.dram_tensor("partial", [P, 1], f32, kind="ExternalOutput").ap()

    with tile.TileContext(nc) as tc:
        with (
            tc.tile_pool(name="tgt_p", bufs=3) as tgt_p,
            tc.tile_pool(name="mean_p", bufs=3) as mean_p,
            tc.tile_pool(name="lstd_p", bufs=3) as lstd_p,
            tc.tile_pool(name="e1_p", bufs=2) as e1_p,
            tc.tile_pool(name="diff_p", bufs=2) as diff_p,
            tc.tile_pool(name="small_p", bufs=3) as small_p,
            tc.tile_pool(name="persist", bufs=1) as persist,
        ):
            t_lmx = persist.tile([P, T * K], f32)
            nc.sync.dma_start(out=t_lmx, in_=lmx)
            t_nm = persist.tile([P, T], f32)      # per-tile -max_k score
            t_sacc = persist.tile([P, T], f32)    # per-tile sum_k exp(score+nm)

            state = {}

            def emit_a(t):
                """Front: DMAs, sls reduce, e1 = exp(-logstd), diff on Pool."""
                rows = slice(t * P, (t + 1) * P)
                t_tgt = tgt_p.tile([P, D], bf16)
                t_mean = mean_p.tile([P, K, D], bf16)
                t_lstd = lstd_p.tile([P, K, D], f8)
                mean3 = mean[rows].rearrange("p (k d) -> p k d", k=K)
                lstd3 = lstd[rows].rearrange("p (k d) -> p k d", k=K)
                if t != 0:
                    nc.sync.dma_start(out=t_lstd, in_=lstd3)
                    nc.sync.dma_start(out=t_tgt, in_=tgt[rows])
                    nc.sync.dma_start(out=t_mean, in_=mean3)
                else:
                    # chunked so first compute starts after ~1/5 of the load
                    nc.sync.dma_start(out=t_lstd[:, 0, :], in_=lstd3[:, 0, :])
                    nc.sync.dma_start(out=t_tgt, in_=tgt[rows])
                    for k in range(1, K):
                        nc.sync.dma_start(out=t_lstd[:, k, :], in_=lstd3[:, k, :])
                    for k in range(K):
                        nc.sync.dma_start(out=t_mean[:, k, :], in_=mean3[:, k, :])

                # sls_k = sum_d logstd: one grouped 3D reduce on DVE
                t_sls = small_p.tile([P, K], f32)
                nc.vector.tensor_reduce(
                    out=t_sls, in_=t_lstd, axis=AX.X, op=AL.add
                )
                # e1 = exp(-logstd) on ACT (fp8 in, bf16 out)
                t_e1 = e1_p.tile([P, K, D], bf16)
                nc.scalar.activation(out=t_e1, in_=t_lstd, func=AF.Exp, scale=-1.0)

                # diff = target (broadcast over k) - mean on DVE (bf16 2x mode)
                t_diff = diff_p.tile([P, K, D], bf16)
                tgt_b = bass.AP(
                    tensor=t_tgt.tensor, offset=t_tgt.offset,
                    ap=[t_tgt.ap[0], [0, K], t_tgt.ap[1]],
                )
                nc.vector.tensor_tensor(
                    out=t_diff, in0=tgt_b, in1=t_mean, op=AL.subtract
                )
                state[t] = (t_diff, t_e1, t_sls)

            def emit_b(t):
                """Back: z, squares w/ accumulate, logsumexp smalls."""
                t_diff, t_e1, t_sls = state.pop(t)
                # z = diff * e1 in place (3D DVE mult, 2x bf16 mode)
                nc.vector.tensor_tensor(out=t_diff, in0=t_diff, in1=t_e1, op=AL.mult)
                t_h = small_p.tile([P, K], f32)
                for k in range(ACT_SQ_K):
                    nc.scalar.activation(
                        out=t_diff[:, k, :], in_=t_diff[:, k, :], func=AF.Square,
                        accum_out=t_h[:, k : k + 1],
                    )
                for k in range(ACT_SQ_K, K):
                    nc.vector.scalar_tensor_tensor(
                        out=t_diff[:, k, :], in0=t_diff[:, k, :], scalar=1.0,
                        in1=t_diff[:, k, :], op0=AL.mult, op1=AL.mult,
                        accum_out=t_h[:, k : k + 1],
                    )

                # score_k = -0.5*h_k - sls_k + lmx_k ; nm = -max_k score
                t_q = small_p.tile([P, K], f32)
                nc.vector.scalar_tensor_tensor(
                    out=t_q, in0=t_h, scalar=-0.5, in1=t_sls,
                    op0=AL.mult, op1=AL.subtract,
                )
                t_c = small_p.tile([P, K], f32)
                nc.vector.tensor_tensor(
                    out=t_c, in0=t_q, in1=t_lmx[:, t * K : (t + 1) * K], op=AL.add
                )
                nc.vector.tensor_reduce(
                    out=t_nm[:, t : t + 1], in_=t_c, axis=AX.X, op=AL.max, negate=True
                )
                # S_t = sum_k exp(score + nm)
                t_e = small_p.tile([P, K], f32)
                nc.scalar.activation(
                    out=t_e, in_=t_c, func=AF.Exp, bias=t_nm[:, t : t + 1],
                    scale=1.0, accum_out=t_sacc[:, t : t + 1],
                )

            # software-pipelined emission: tile t+1's front stage is queued
            # before tile t's back stage
            emit_a(0)
            for t in range(T):
                if t + 1 < T:
                    emit_a(t + 1)
                emit_b(t)

            # loss rows: nm_t - ln(S_t); single Ln pass keeps Ln out of the loop
            t_lns = persist.tile([P, T], f32)
            nc.scalar.activation(out=t_lns, in_=t_sacc, func=AF.Ln)
            t_accv = persist.tile([P, T], f32)
            nc.vector.tensor_tensor(out=t_accv, in0=t_nm, in1=t_lns, op=AL.subtract)
            t_tot = persist.tile([P, 1], f32)
            nc.vector.tensor_reduce(out=t_tot, in_=t_accv, axis=AX.X, op=AL.add)
            nc.sync.dma_start(out=out, in_=t_tot)

    nc.compile()
    return nc


def get_nc():
    global _NC
    if _NC is None:
        _NC = _build()
    return _NC


def make_in_maps(target, s_mean, s_logstd, log_mix_coeffs):
    target = np.asarray(target, dtype=np.float32).astype(ml_dtypes.bfloat16)
    s_mean = np.asarray(s_mean, dtype=np.float32).astype(ml_dtypes.bfloat16)
    s_logstd = np.asarray(s_logstd, dtype=np.float32).astype(ml_dtypes.float8_e4m3)
    lm = np.ascontiguousarray(np.asarray(log_mix_coeffs, dtype=np.float32))
    in_maps = []
    for c in range(NCORES):
        rows = slice(c * R, (c + 1) * R)
        # pack log-mix so tile t's [128, K] block sits at columns [t*K, (t+1)*K)
        lmx = lm[rows].reshape(T, P, K).transpose(1, 0, 2).reshape(P, T * K)
        in_maps.append({
            "tgt": np.ascontiguousarray(target[rows]),
            "mean": np.ascontiguousarray(s_mean[rows]),
            "lstd": np.ascontiguousarray(s_logstd[rows]),
            "lmx": np.ascontiguousarray(lmx),
        })
    return in_maps


def combine(results):
    total = sum(float(np.asarray(r["partial"], dtype=np.float64).sum()) for r in results)
    return np.float32(total / N)


def kernel(target, s_mean, s_logstd, log_mix_coeffs):
    from concourse.bass_utils import run_bass_kernel_spmd

    nc = get_nc()
    in_maps = make_in_maps(target, s_mean, s_logstd, log_mix_coeffs)
    res = run_bass_kernel_spmd(nc, in_maps, core_ids=list(range(NCORES)))
    return combine(res.results)


# revision 6
# speedup vs baseline: 1.5963x; 1.0077x over previous
"""MDN-RNN loss kernel v4: transposed layout, PE-driven reductions.

Layout (per core, R=2048 rows): host ships tensors TRANSPOSED so the
feature dim D sits on partitions and the token rows sit on the free dim:
    tgtT  [D, R]        bf16   (chunks c: [128, R] x8 + [64, R])
    meanT [D, K, R]     bf16   (chunk c rows c*128.., k middle)
    lstdT [D, K, R]     fp8e4
    lmx   [P, T*K]      f32    (row-major packing for the logsumexp tail)

Per chunk c (free size 2048 per k-slice):
    ACT:  e1 = exp(-lstd) (one [Pc,K,R] pass), Square for ~55% of k-slices
    DVE:  diff_k = tgt - mean_k (per-k tt, bf16 2x), z = diff*e1 (3D tt),
          w = z*z for the remaining k-slices
    PE :  h_k   += ones^T @ w_k   (accumulates over c into PSUM [K, R])
          sls_k += ones^T @ lstd_k
Tail: score1 = -0.5*h - sls (DVE stt from PSUM), PE-transpose score1 back
to row-major [128, T, K], + log_mix, then the standard stable logsumexp
smalls and the [P,1] partial-sum output.
"""

import sys

if "/opt/trn_rl_repo" not in sys.path:
    sys.path.insert(0, "/opt/trn_rl_repo")

import numpy as np
import ml_dtypes

N = 16384
K = 5
D = 1088
KD = K * D
NCORES = 8
R = N // NCORES          # 2048 rows per core
P = 128                  # partitions
T = R // P               # 16 row-tiles (tail packing)
NC_FULL = D // P         # 8 full chunks
TAILP = D - NC_FULL * P  # 64
NCHUNK = NC_FULL + 1     # 9

# per-chunk count of k-slices whose square runs on ACT (rest: DVE w=z*z)
ACT_SQ = {c: (3 if c < 7 else 2) for c in range(NCHUNK)}

_NC = None


def _build():
    import concourse.bacc as bacc
    import concourse.bass as bass
    import concourse.tile as tile
    from concourse import mybir

    AF = mybir.ActivationFunctionType
    AL = mybir.AluOpType
    AX = mybir.AxisListType
    f32 = mybir.dt.float32
    bf16 = mybir.dt.bfloat16
    f8 = mybir.dt.float8e4

    nc = bacc.Bacc("TRN2", debug=False)
    tgt = nc.dram_tensor("tgt", [D, R], bf16, kind="ExternalInput").ap()
    mean = nc.dram_tensor("mean", [D, K, R], bf16, kind="ExternalInput").ap()
    lstd = nc.dram_tensor("lstd", [D, K, R], f8, kind="ExternalInput").ap()
    lmx = nc.dram_tensor("lmx", [P, T * K], f32, kind="ExternalInput").ap()
    ident_d = nc.dram_tensor("ident", [K, K], f32, kind="ExternalInput").ap()
    out = nc.dram_tensor("partial", [P, 1], f32, kind="ExternalOutput").ap()

    with tile.TileContext(nc) as tc:
        with (
            tc.tile_pool(name="tgt_p", bufs=1) as tgt_p,
            tc.tile_pool(name="mean_p", bufs=2) as mean_p,
            tc.tile_pool(name="lstd_p", bufs=2) as lstd_p,
            tc.tile_pool(name="e1_p", bufs=2) as e1_p,
            tc.tile_pool(name="ones_p", bufs=1) as ones_p,
            tc.tile_pool(name="small_p", bufs=2) as small_p,
            tc.tile_pool(name="persist", bufs=1) as persist,
            tc.tile_pool(name="psum_h", bufs=1, space="PSUM") as psum_h,
            tc.tile_pool(name="psum_s", bufs=1, space="PSUM") as psum_s,
        ):
            t_lmx = persist.tile([P, T * K], f32)
            nc.sync.dma_start(out=t_lmx, in_=lmx)

            ones_bf = ones_p.tile([P, 1], bf16)
            nc.vector.memset(ones_bf, 1.0)
            twos_f8 = ones_p.tile([P, 1], f8)
            nc.vector.memset(twos_f8, 2.0)
            # 5x5 identity for the PE transpose tail (memset can't write at
            # partition bases > 0, so DMA it in)
            ident = ones_p.tile([K, K], f32)
            nc.sync.dma_start(out=ident, in_=ident_d)

            # whole-core resident target (36 KB/partition)
            t_tgt = tgt_p.tile([P, NCHUNK, R], bf16)

            # merged accumulators v_k = sum_d z^2 + 2*sum_d logstd, one PSUM
            # row per k at matmul-legal base partitions {0,32,64,96} + {0}
            vA = psum_h.tile([P, R], f32)       # k=0,1,2 at partitions 0/32/64
            vB = psum_s.tile([P, R], f32)       # k=3,4 at partitions 0/32

            def v_row(k):
                if k < 3:
                    return vA[k * 32 : k * 32 + 1, :]
                return vB[(k - 3) * 32 : (k - 3) * 32 + 1, :]

            state = {}

            def pc_of(c):
                return P if c < NC_FULL else TAILP

            def emit_a(c):
                pc = pc_of(c)
                rows = slice(c * P, c * P + pc)
                t_mean = mean_p.tile([P, K, R], bf16)
                t_lstd = lstd_p.tile([P, K, R], f8)
                nc.sync.dma_start(out=t_lstd[:pc], in_=lstd[rows])
                if c == 0:
                    for cc in range(NCHUNK):
                        pcc = pc_of(cc)
                        nc.sync.dma_start(
                            out=t_tgt[:pcc, cc, :],
                            in_=tgt[cc * P : cc * P + pcc],
                        )
                nc.sync.dma_start(out=t_mean[:pc], in_=mean[rows])

                # e1 = exp(-lstd), one 3D pass on ACT
                t_e1 = e1_p.tile([P, K, R], bf16)
                nc.scalar.activation(
                    out=t_e1[:pc], in_=t_lstd[:pc], func=AF.Exp, scale=-1.0
                )
                # 2*sum(logstd) matmuls on PE (fp8 twos stationary, exact);
                # one matmul per 512-col block (PSUM-bank ISA limit)
                for k in range(K):
                    vr = v_row(k)
                    for b in range(0, R, 512):
                        nc.tensor.matmul(
                            vr[:, b : b + 512],
                            twos_f8[:pc],
                            t_lstd[:pc, k, b : b + 512],
                            start=(c == 0),
                            stop=False,
                            skip_group_check=True,
                        )
                # diff_k = tgt_c - mean_k, per-k 2D tt (keeps bf16 2x mode)
                for k in range(K):
                    nc.vector.tensor_tensor(
                        out=t_mean[:pc, k, :], in0=t_tgt[:pc, c, :],
                        in1=t_mean[:pc, k, :], op=AL.subtract,
                    )
                state[c] = (t_mean, t_e1)

            def emit_b(c):
                pc = pc_of(c)
                t_mean, t_e1 = state.pop(c)
                # z = diff * e1 in place (3D tt, bf16 2x)
                nc.vector.tensor_tensor(
                    out=t_mean[:pc], in0=t_mean[:pc], in1=t_e1[:pc], op=AL.mult
                )
                a_sq = ACT_SQ[c]
                for k in range(K):
                    zk = t_mean[:pc, k, :]
                    wk = t_e1[:pc, k, :]      # e1 slice is dead after z
                    if k < a_sq:
                        nc.scalar.activation(out=wk, in_=zk, func=AF.Square)
                    else:
                        nc.vector.tensor_tensor(out=wk, in0=zk, in1=zk, op=AL.mult)
                    vr = v_row(k)
                    for b in range(0, R, 512):
                        nc.tensor.matmul(
                            vr[:, b : b + 512],
                            ones_bf[:pc],
                            wk[:, b : b + 512],
                            start=False,
                            stop=(c == NCHUNK - 1),
                            skip_group_check=True,
                        )

            emit_a(0)
            for c in range(NCHUNK):
                if c + 1 < NCHUNK:
                    emit_a(c + 1)
                emit_b(c)

            # ---- tail ----
            # PSUM -> SBUF stage (ACT, partition-aligned), then repack the
            # five rows onto partitions 0..4 with single-row SBUF DMAs
            t_stage = persist.tile([P, 2, R], f32)
            for k in range(K):
                a, b = (k, 0) if k < 3 else (k - 3, 1)
                nc.scalar.activation(
                    out=t_stage[a * 32 : a * 32 + 1, b, :], in_=v_row(k),
                    func=AF.Copy,
                )
            t_sc1 = persist.tile([K, R], f32)
            for k in range(K):
                a, b = (k, 0) if k < 3 else (k - 3, 1)
                nc.sync.dma_start(
                    out=t_sc1[k : k + 1, :],
                    in_=t_stage[a * 32 : a * 32 + 1, b, :],
                )
            # transpose [K, R] -> row-major [128, T, K] via 16 PE transposes,
            # reusing vA's (now dead) PSUM banks as the destination
            sc_ps = vA[:, 0 : T * K].rearrange("p (t k) -> p t k", k=K)
            for t in range(T):
                nc.tensor.transpose(
                    sc_ps[:, t, :], t_sc1[:, t * P : (t + 1) * P], ident
                )
            # score = -0.5*v + log_mix (row-major packing)
            t_sc2 = persist.tile([P, T, K], f32)
            nc.vector.scalar_tensor_tensor(
                out=t_sc2, in0=sc_ps, scalar=-0.5,
                in1=t_lmx.rearrange("p (t k) -> p t k", k=K),
                op0=AL.mult, op1=AL.add,
            )
            t_nm = persist.tile([P, T], f32)
            nc.vector.tensor_reduce(
                out=t_nm, in_=t_sc2, axis=AX.X, op=AL.max, negate=True
            )
            nm_b = bass.AP(
                tensor=t_nm.tensor, offset=t_nm.offset,
                ap=[t_nm.ap[0], t_nm.ap[1], [0, K]],
            )
            t_es = persist.tile([P, T, K], f32)
            nc.vector.tensor_tensor(out=t_es, in0=t_sc2, in1=nm_b, op=AL.add)
            t_ex = persist.tile([P, T, K], f32)
            nc.scalar.activation(out=t_ex, in_=t_es, func=AF.Exp)
            t_S = persist.tile([P, T], f32)
            nc.vector.tensor_reduce(out=t_S, in_=t_ex, axis=AX.X, op=AL.add)
            t_lns = persist.tile([P, T], f32)
            nc.scalar.activation(out=t_lns, in_=t_S, func=AF.Ln)
            t_accv = persist.tile([P, T], f32)
            nc.vector.tensor_tensor(out=t_accv, in0=t_nm, in1=t_lns, op=AL.subtract)
            t_tot = persist.tile([P, 1], f32)
            nc.vector.tensor_reduce(out=t_tot, in_=t_accv, axis=AX.X, op=AL.add)
            nc.sync.dma_start(out=out, in_=t_tot)

    nc.compile()
    return nc


def get_nc():
    global _NC
    if _NC is None:
        _NC = _build()
    return _NC


def make_in_maps(target, s_mean, s_logstd, log_mix_coeffs):
    target = np.asarray(target, dtype=np.float32)
    s_mean = np.asarray(s_mean, dtype=np.float32)
    s_logstd = np.asarray(s_logstd, dtype=np.float32)
    lm = np.asarray(log_mix_coeffs, dtype=np.float32)
    in_maps = []
    for c in range(NCORES):
        rows = slice(c * R, (c + 1) * R)
        tgtT = np.ascontiguousarray(target[rows].T.astype(ml_dtypes.bfloat16))
        meanT = np.ascontiguousarray(
            s_mean[rows].reshape(R, K, D).transpose(2, 1, 0)
            .astype(ml_dtypes.bfloat16)
        )
        lstdT = np.ascontiguousarray(
            s_logstd[rows].reshape(R, K, D).transpose(2, 1, 0)
            .astype(ml_dtypes.float8_e4m3)
        )
        lmx = np.ascontiguousarray(
            lm[rows].reshape(T, P, K).transpose(1, 0, 2).reshape(P, T * K)
        )
        in_maps.append({
            "tgt": tgtT, "mean": meanT, "lstd": lstdT, "lmx": lmx,
            "ident": np.eye(K, dtype=np.float32),
        })
    return in_maps


def combine(results):
    total = sum(float(np.asarray(r["partial"], dtype=np.float64).sum()) for r in results)
    return np.float32(total / N)


def kernel(target, s_mean, s_logstd, log_mix_coeffs):
    from concourse.bass_utils import run_bass_kernel_spmd

    nc = get_nc()
    in_maps = make_in_maps(target, s_mean, s_logstd, log_mix_coeffs)
    res = run_bass_kernel_spmd(nc, in_maps, core_ids=list(range(NCORES)))
    return combine(res.results)


# revision 7
# speedup vs baseline: 1.6124x; 1.0101x over previous
"""MDN-RNN loss kernel v4: transposed layout, PE-driven reductions.

Layout (per core, R=2048 rows): host ships tensors TRANSPOSED so the
feature dim D sits on partitions and the token rows sit on the free dim:
    tgtT  [D, R]        bf16   (chunks c: [128, R] x8 + [64, R])
    meanT [D, K, R]     bf16   (chunk c rows c*128.., k middle)
    lstdT [D, K, R]     fp8e4
    lmx   [P, T*K]      f32    (row-major packing for the logsumexp tail)

Per chunk c (free size 2048 per k-slice):
    ACT:  e1 = exp(-lstd) (one [Pc,K,R] pass), Square for ~55% of k-slices
    DVE:  diff_k = tgt - mean_k (per-k tt, bf16 2x), z = diff*e1 (3D tt),
          w = z*z for the remaining k-slices
    PE :  h_k   += ones^T @ w_k   (accumulates over c into PSUM [K, R])
          sls_k += ones^T @ lstd_k
Tail: score1 = -0.5*h - sls (DVE stt from PSUM), PE-transpose score1 back
to row-major [128, T, K], + log_mix, then the standard stable logsumexp
smalls and the [P,1] partial-sum output.
"""

import sys

if "/opt/trn_rl_repo" not in sys.path:
    sys.path.insert(0, "/opt/trn_rl_repo")

import numpy as np
import ml_dtypes

N = 16384
K = 5
D = 1088
KD = K * D
NCORES = 8
R = N // NCORES          # 2048 rows per core
P = 128                  # partitions
T = R // P               # 16 row-tiles (tail packing)
NC_FULL = D // P         # 8 full chunks
TAILP = D - NC_FULL * P  # 64
NCHUNK = NC_FULL + 1     # 9

# per-chunk count of k-slices whose square runs on ACT (rest: DVE w=z*z)
ACT_SQ = {c: (3 if c < 4 else 2) for c in range(NCHUNK)}

_NC = None


def _build():
    import concourse.bacc as bacc
    import concourse.bass as bass
    import concourse.tile as tile
    from concourse import mybir

    AF = mybir.ActivationFunctionType
    AL = mybir.AluOpType
    AX = mybir.AxisListType
    f32 = mybir.dt.float32
    bf16 = mybir.dt.bfloat16
    f8 = mybir.dt.float8e4

    nc = bacc.Bacc("TRN2", debug=False)
    tgt = nc.dram_tensor("tgt", [D, R], bf16, kind="ExternalInput").ap()
    mean = nc.dram_tensor("mean", [D, K, R], bf16, kind="ExternalInput").ap()
    lstd = nc.dram_tensor("lstd", [D, K, R], f8, kind="ExternalInput").ap()
    lmx = nc.dram_tensor("lmx", [P, T * K], f32, kind="ExternalInput").ap()
    ident_d = nc.dram_tensor("ident", [K, K], f32, kind="ExternalInput").ap()
    out = nc.dram_tensor("partial", [P, 1], f32, kind="ExternalOutput").ap()

    with tile.TileContext(nc) as tc:
        with (
            tc.tile_pool(name="tgt_p", bufs=1) as tgt_p,
            tc.tile_pool(name="mean_p", bufs=2) as mean_p,
            tc.tile_pool(name="lstd_p", bufs=4) as lstd_p,
            tc.tile_pool(name="e1_p", bufs=2) as e1_p,
            tc.tile_pool(name="ones_p", bufs=1) as ones_p,
            tc.tile_pool(name="small_p", bufs=2) as small_p,
            tc.tile_pool(name="persist", bufs=1) as persist,
            tc.tile_pool(name="psum_h", bufs=1, space="PSUM") as psum_h,
            tc.tile_pool(name="psum_s", bufs=1, space="PSUM") as psum_s,
        ):
            t_lmx = persist.tile([P, T * K], f32)
            nc.sync.dma_start(out=t_lmx, in_=lmx)

            ones_bf = ones_p.tile([P, 1], bf16)
            nc.vector.memset(ones_bf, 1.0)
            twos_f8 = ones_p.tile([P, 1], f8)
            nc.vector.memset(twos_f8, 2.0)
            # 5x5 identity for the PE transpose tail (memset can't write at
            # partition bases > 0, so DMA it in)
            ident = ones_p.tile([K, K], f32)
            nc.sync.dma_start(out=ident, in_=ident_d)

            # whole-core resident target (36 KB/partition)
            t_tgt = tgt_p.tile([P, NCHUNK, R], bf16)

            # merged accumulators v_k = sum_d z^2 + 2*sum_d logstd, one PSUM
            # row per k at matmul-legal base partitions {0,32,64,96} + {0}
            vA = psum_h.tile([P, R], f32)       # k=0,1,2 at partitions 0/32/64
            vB = psum_s.tile([P, R], f32)       # k=3,4 at partitions 0/32

            def v_row(k):
                if k < 3:
                    return vA[k * 32 : k * 32 + 1, :]
                return vB[(k - 3) * 32 : (k - 3) * 32 + 1, :]

            state = {}

            def pc_of(c):
                return P if c < NC_FULL else TAILP

            lstate = {}

            def emit_sls(c):
                """DMA logstd + its 20 PE matmuls: runs chunks ahead of the
                main stage so the PE queue never drains (keeps the tensor
                engine past its 3us continuous-execution ramp threshold)."""
                pc = pc_of(c)
                rows = slice(c * P, c * P + pc)
                t_lstd = lstd_p.tile([P, K, R], f8)
                t_mean = mean_p.tile([P, K, R], bf16)
                nc.sync.dma_start(out=t_lstd[:pc], in_=lstd[rows])
                nc.sync.dma_start(out=t_mean[:pc], in_=mean[rows])
                if c == 0:
                    for cc in range(NCHUNK):
                        pcc = pc_of(cc)
                        nc.sync.dma_start(
                            out=t_tgt[:pcc, cc, :],
                            in_=tgt[cc * P : cc * P + pcc],
                        )
                # 2*sum(logstd) matmuls on PE (fp8 twos stationary, exact);
                # one matmul per 512-col block (PSUM-bank ISA limit)
                for k in range(K):
                    vr = v_row(k)
                    for b in range(0, R, 512):
                        nc.tensor.matmul(
                            vr[:, b : b + 512],
                            twos_f8[:pc],
                            t_lstd[:pc, k, b : b + 512],
                            start=(c == 0),
                            stop=False,
                            skip_group_check=True,
                        )
                lstate[c] = (t_lstd, t_mean)

            def emit_a(c):
                pc = pc_of(c)
                t_lstd, t_mean = lstate.pop(c)

                # e1 = exp(-lstd), one 3D pass on ACT
                t_e1 = e1_p.tile([P, K, R], bf16)
                nc.scalar.activation(
                    out=t_e1[:pc], in_=t_lstd[:pc], func=AF.Exp, scale=-1.0
                )
                # diff_k = tgt_c - mean_k, per-k 2D tt (keeps bf16 2x mode)
                for k in range(K):
                    nc.vector.tensor_tensor(
                        out=t_mean[:pc, k, :], in0=t_tgt[:pc, c, :],
                        in1=t_mean[:pc, k, :], op=AL.subtract,
                    )
                state[c] = (t_mean, t_e1)

            def emit_b(c):
                pc = pc_of(c)
                t_mean, t_e1 = state.pop(c)
                a_sq = ACT_SQ[c]
                for k in range(K):
                    # per-k z so the first square (and its PE matmuls)
                    # releases after ~1.2us instead of after the whole 3D z
                    nc.vector.tensor_tensor(
                        out=t_mean[:pc, k, :], in0=t_mean[:pc, k, :],
                        in1=t_e1[:pc, k, :], op=AL.mult,
                    )
                    zk = t_mean[:pc, k, :]
                    wk = t_e1[:pc, k, :]      # e1 slice is dead after z
                    if k < a_sq:
                        nc.scalar.activation(out=wk, in_=zk, func=AF.Square)
                    else:
                        nc.vector.tensor_tensor(out=wk, in0=zk, in1=zk, op=AL.mult)
                    vr = v_row(k)
                    for b in range(0, R, 512):
                        nc.tensor.matmul(
                            vr[:, b : b + 512],
                            ones_bf[:pc],
                            wk[:, b : b + 512],
                            start=False,
                            stop=(c == NCHUNK - 1),
                            skip_group_check=True,
                        )

            # exp(c+1) is emitted AFTER emit_b(c) so the ACT queue never
            # head-of-line-blocks chunk c's squares behind the next exp
            emit_sls(0)
            emit_sls(1)
            emit_sls(2)
            emit_a(0)
            for c in range(NCHUNK):
                if c + 3 < NCHUNK:
                    emit_sls(c + 3)
                emit_b(c)
                if c + 1 < NCHUNK:
                    emit_a(c + 1)

            # ---- tail ----
            # PSUM -> SBUF stage (ACT, partition-aligned), then repack the
            # five rows onto partitions 0..4 with single-row SBUF DMAs
            t_stage = persist.tile([P, 2, R], f32)
            for k in range(K):
                a, b = (k, 0) if k < 3 else (k - 3, 1)
                nc.scalar.activation(
                    out=t_stage[a * 32 : a * 32 + 1, b, :], in_=v_row(k),
                    func=AF.Copy,
                )
            t_sc1 = persist.tile([K, R], f32)
            for k in range(K):
                a, b = (k, 0) if k < 3 else (k - 3, 1)
                nc.sync.dma_start(
                    out=t_sc1[k : k + 1, :],
                    in_=t_stage[a * 32 : a * 32 + 1, b, :],
                )
            # transpose [K, R] -> row-major [128, T, K] via 16 PE transposes,
            # reusing vA's (now dead) PSUM banks as the destination
            sc_ps = vA[:, 0 : T * K].rearrange("p (t k) -> p t k", k=K)
            for t in range(T):
                nc.tensor.transpose(
                    sc_ps[:, t, :], t_sc1[:, t * P : (t + 1) * P], ident
                )
            # score = -0.5*v + log_mix (row-major packing)
            t_sc2 = persist.tile([P, T, K], f32)
            nc.vector.scalar_tensor_tensor(
                out=t_sc2, in0=sc_ps, scalar=-0.5,
                in1=t_lmx.rearrange("p (t k) -> p t k", k=K),
                op0=AL.mult, op1=AL.add,
            )
            t_nm = persist.tile([P, T], f32)
            nc.vector.tensor_reduce(
                out=t_nm, in_=t_sc2, axis=AX.X, op=AL.max, negate=True
            )
            nm_b = bass.AP(
                tensor=t_nm.tensor, offset=t_nm.offset,
                ap=[t_nm.ap[0], t_nm.ap[1], [0, K]],
            )
            t_es = persist.tile([P, T, K], f32)
            nc.vector.tensor_tensor(out=t_es, in0=t_sc2, in1=nm_b, op=AL.add)
            t_ex = persist.tile([P, T, K], f32)
            nc.scalar.activation(out=t_ex, in_=t_es, func=AF.Exp)
            t_S = persist.tile([P, T], f32)
            nc.vector.tensor_reduce(out=t_S, in_=t_ex, axis=AX.X, op=AL.add)
            t_lns = persist.tile([P, T], f32)
            nc.scalar.activation(out=t_lns, in_=t_S, func=AF.Ln)
            t_accv = persist.tile([P, T], f32)
            nc.vector.tensor_tensor(out=t_accv, in0=t_nm, in1=t_lns, op=AL.subtract)
            t_tot = persist.tile([P, 1], f32)
            nc.vector.tensor_reduce(out=t_tot, in_=t_accv, axis=AX.X, op=AL.add)
            nc.sync.dma_start(out=out, in_=t_tot)

    nc.compile()
    return nc


def get_nc():
    global _NC
    if _NC is None:
        _NC = _build()
    return _NC


def make_in_maps(target, s_mean, s_logstd, log_mix_coeffs):
    target = np.asarray(target, dtype=np.float32)
    s_mean = np.asarray(s_mean, dtype=np.float32)
    s_logstd = np.asarray(s_logstd, dtype=np.float32)
    lm = np.asarray(log_mix_coeffs, dtype=np.float32)
    in_maps = []
    for c in range(NCORES):
        rows = slice(c * R, (c + 1) * R)
        tgtT = np.ascontiguousarray(target[rows].T.astype(ml_dtypes.bfloat16))
        meanT = np.ascontiguousarray(
            s_mean[rows].reshape(R, K, D).transpose(2, 1, 0)
            .astype(ml_dtypes.bfloat16)
        )
        lstdT = np.ascontiguousarray(
            s_logstd[rows].reshape(R, K, D).transpose(2, 1, 0)
            .astype(ml_dtypes.float8_e4m3)
        )
        lmx = np.ascontiguousarray(
            lm[rows].reshape(T, P, K).transpose(1, 0, 2).reshape(P, T * K)
        )
        in_maps.append({
            "tgt": tgtT, "mean": meanT, "lstd": lstdT, "lmx": lmx,
            "ident": np.eye(K, dtype=np.float32),
        })
    return in_maps


def combine(results):
    total = sum(float(np.asarray(r["partial"], dtype=np.float64).sum()) for r in results)
    return np.float32(total / N)


def kernel(target, s_mean, s_logstd, log_mix_coeffs):
    from concourse.bass_utils import run_bass_kernel_spmd

    nc = get_nc()
    in_maps = make_in_maps(target, s_mean, s_logstd, log_mix_coeffs)
    res = run_bass_kernel_spmd(nc, in_maps, core_ids=list(range(NCORES)))
    return combine(res.results)


# revision 8
# speedup vs baseline: 1.6813x; 1.0427x over previous
"""MDN-RNN loss kernel v4: transposed layout, PE-driven reductions.

Layout (per core, R=2048 rows): host ships tensors TRANSPOSED so the
feature dim D sits on partitions and the token rows sit on the free dim:
    tgtT  [D, R]        bf16   (chunks c: [128, R] x8 + [64, R])
    meanT [D, K, R]     bf16   (chunk c rows c*128.., k middle)
    lstdT [D, K, R]     fp8e4
    lmx   [P, T*K]      f32    (row-major packing for the logsumexp tail)

Per chunk c (free size 2048 per k-slice):
    ACT:  e1 = exp(-lstd) (one [Pc,K,R] pass), Square for ~55% of k-slices
    DVE:  diff_k = tgt - mean_k (per-k tt, bf16 2x), z = diff*e1 (3D tt),
          w = z*z for the remaining k-slices
    PE :  h_k   += ones^T @ w_k   (accumulates over c into PSUM [K, R])
          sls_k += ones^T @ lstd_k
Tail: score1 = -0.5*h - sls (DVE stt from PSUM), PE-transpose score1 back
to row-major [128, T, K], + log_mix, then the standard stable logsumexp
smalls and the [P,1] partial-sum output.
"""

import sys

if "/opt/trn_rl_repo" not in sys.path:
    sys.path.insert(0, "/opt/trn_rl_repo")

import numpy as np
import ml_dtypes

N = 16384
K = 5
D = 1088
KD = K * D
NCORES = 8
R = N // NCORES          # 2048 rows per core
P = 128                  # partitions
T = R // P               # 16 row-tiles (tail packing)
NC_FULL = D // P         # 8 full chunks
TAILP = D - NC_FULL * P  # 64
NCHUNK = NC_FULL + 1     # 9

# per-chunk count of k-slices whose square runs on ACT (rest: DVE w=z*z)
ACT_SQ = {c: 2 for c in range(NCHUNK)}

_NC = None


def _build():
    import concourse.bacc as bacc
    import concourse.bass as bass
    import concourse.tile as tile
    from concourse import mybir

    AF = mybir.ActivationFunctionType
    AL = mybir.AluOpType
    AX = mybir.AxisListType
    f32 = mybir.dt.float32
    bf16 = mybir.dt.bfloat16
    f8 = mybir.dt.float8e4

    nc = bacc.Bacc("TRN2", debug=False)
    tgt = nc.dram_tensor("tgt", [D, R], bf16, kind="ExternalInput").ap()
    mean = nc.dram_tensor("mean", [D, K, R], bf16, kind="ExternalInput").ap()
    lstd = nc.dram_tensor("lstd", [D, K, R], f8, kind="ExternalInput").ap()
    lmx = nc.dram_tensor("lmx", [P, T * K], f32, kind="ExternalInput").ap()
    ident_d = nc.dram_tensor("ident", [K, K], f32, kind="ExternalInput").ap()
    out = nc.dram_tensor("partial", [P, 1], f32, kind="ExternalOutput").ap()

    with tile.TileContext(nc) as tc:
        with (
            tc.tile_pool(name="tgt_p", bufs=1) as tgt_p,
            tc.tile_pool(name="mean_p", bufs=2) as mean_p,
            tc.tile_pool(name="lstd_p", bufs=4) as lstd_p,
            tc.tile_pool(name="e1_p", bufs=2) as e1_p,
            tc.tile_pool(name="ones_p", bufs=1) as ones_p,
            tc.tile_pool(name="small_p", bufs=2) as small_p,
            tc.tile_pool(name="persist", bufs=1) as persist,
            tc.tile_pool(name="psum_h", bufs=1, space="PSUM") as psum_h,
            tc.tile_pool(name="psum_s", bufs=1, space="PSUM") as psum_s,
        ):
            t_lmx = persist.tile([P, T * K], f32)
            nc.sync.dma_start(out=t_lmx, in_=lmx)

            ones_bf = ones_p.tile([P, 1], bf16)
            nc.vector.memset(ones_bf, 1.0)
            twos_f8 = ones_p.tile([P, 1], f8)
            nc.vector.memset(twos_f8, 2.0)
            # 5x5 identity for the PE transpose tail (memset can't write at
            # partition bases > 0, so DMA it in)
            ident = ones_p.tile([K, K], f32)
            nc.sync.dma_start(out=ident, in_=ident_d)

            # whole-core resident target (36 KB/partition)
            t_tgt = tgt_p.tile([P, NCHUNK, R], bf16)

            # merged accumulators v_k = sum_d z^2 + 2*sum_d logstd, one PSUM
            # row per k at matmul-legal base partitions {0,32,64,96} + {0}
            vA = psum_h.tile([P, R], f32)       # k=0,1,2 at partitions 0/32/64
            vB = psum_s.tile([P, R], f32)       # k=3,4 at partitions 0/32

            def v_row(k):
                if k < 3:
                    return vA[k * 32 : k * 32 + 1, :]
                return vB[(k - 3) * 32 : (k - 3) * 32 + 1, :]

            state = {}

            def pc_of(c):
                return P if c < NC_FULL else TAILP

            lstate = {}

            def emit_sls(c):
                """DMA logstd + its 20 PE matmuls: runs chunks ahead of the
                main stage so the PE queue never drains (keeps the tensor
                engine past its 3us continuous-execution ramp threshold)."""
                pc = pc_of(c)
                rows = slice(c * P, c * P + pc)
                t_lstd = lstd_p.tile([P, K, R], f8)
                t_mean = mean_p.tile([P, K, R], bf16)
                if c == 0:
                    # chunked startup: first sls matmuls + exp fire after
                    # ~1/5 of the load instead of the whole 1.25 MB
                    for k in range(K):
                        nc.sync.dma_start(
                            out=t_lstd[:pc, k, :], in_=lstd[rows][:, k, :]
                        )
                    nc.sync.dma_start(
                        out=t_tgt[:pc, 0, :], in_=tgt[0:pc]
                    )
                    for k in range(K):
                        nc.sync.dma_start(
                            out=t_mean[:pc, k, :], in_=mean[rows][:, k, :]
                        )
                    for cc in range(1, NCHUNK):
                        pcc = pc_of(cc)
                        nc.sync.dma_start(
                            out=t_tgt[:pcc, cc, :],
                            in_=tgt[cc * P : cc * P + pcc],
                        )
                else:
                    nc.sync.dma_start(out=t_lstd[:pc], in_=lstd[rows])
                    nc.sync.dma_start(out=t_mean[:pc], in_=mean[rows])
                # 2*sum(logstd) matmuls on PE (fp8 twos stationary, exact);
                # one matmul per 512-col block (PSUM-bank ISA limit)
                for k in range(K):
                    vr = v_row(k)
                    for b in range(0, R, 512):
                        nc.tensor.matmul(
                            vr[:, b : b + 512],
                            twos_f8[:pc],
                            t_lstd[:pc, k, b : b + 512],
                            start=(c == 0),
                            stop=False,
                            skip_group_check=True,
                        )
                lstate[c] = (t_lstd, t_mean)

            def emit_a(c):
                pc = pc_of(c)
                t_lstd, t_mean = lstate.pop(c)

                # e1 = exp(-lstd), one 3D pass on ACT
                t_e1 = e1_p.tile([P, K, R], bf16)
                nc.scalar.activation(
                    out=t_e1[:pc], in_=t_lstd[:pc], func=AF.Exp, scale=-1.0
                )
                # diff_k = tgt_c - mean_k, per-k 2D tt (keeps bf16 2x mode)
                for k in range(K):
                    nc.vector.tensor_tensor(
                        out=t_mean[:pc, k, :], in0=t_tgt[:pc, c, :],
                        in1=t_mean[:pc, k, :], op=AL.subtract,
                    )
                state[c] = (t_mean, t_e1)

            def emit_b(c):
                pc = pc_of(c)
                t_mean, t_e1 = state.pop(c)
                a_sq = ACT_SQ[c]
                for k in range(K):
                    # per-k z so the first square (and its PE matmuls)
                    # releases after ~1.2us instead of after the whole 3D z
                    nc.vector.tensor_tensor(
                        out=t_mean[:pc, k, :], in0=t_mean[:pc, k, :],
                        in1=t_e1[:pc, k, :], op=AL.mult,
                    )
                    zk = t_mean[:pc, k, :]
                    wk = t_e1[:pc, k, :]      # e1 slice is dead after z
                    if k < a_sq:
                        nc.scalar.activation(out=wk, in_=zk, func=AF.Square)
                    else:
                        nc.vector.tensor_tensor(out=wk, in0=zk, in1=zk, op=AL.mult)
                    vr = v_row(k)
                    for b in range(0, R, 512):
                        nc.tensor.matmul(
                            vr[:, b : b + 512],
                            ones_bf[:pc],
                            wk[:, b : b + 512],
                            start=False,
                            stop=(c == NCHUNK - 1),
                            skip_group_check=True,
                        )

            # exp(c+1) is emitted AFTER emit_b(c) so the ACT queue never
            # head-of-line-blocks chunk c's squares behind the next exp
            emit_sls(0)
            emit_sls(1)
            emit_sls(2)
            emit_a(0)
            for c in range(NCHUNK):
                if c + 3 < NCHUNK:
                    emit_sls(c + 3)
                emit_b(c)
                if c + 1 < NCHUNK:
                    emit_a(c + 1)

            # ---- tail ----
            # PSUM -> SBUF stage (ACT, partition-aligned), then repack the
            # five rows onto partitions 0..4 with single-row SBUF DMAs
            t_stage = persist.tile([P, 2, R], f32)
            for k in range(K):
                a, b = (k, 0) if k < 3 else (k - 3, 1)
                dst = t_stage[a * 32 : a * 32 + 1, b, :]
                nc.scalar.activation(out=dst, in_=v_row(k), func=AF.Copy)
            t_sc1 = persist.tile([K, R], f32)
            for k in range(K):
                a, b = (k, 0) if k < 3 else (k - 3, 1)
                nc.sync.dma_start(
                    out=t_sc1[k : k + 1, :],
                    in_=t_stage[a * 32 : a * 32 + 1, b, :],
                )
            # transpose [K, R] -> row-major [128, T, K] via 16 PE transposes,
            # reusing vA's (now dead) PSUM banks as the destination
            sc_ps = vA[:, 0 : T * K].rearrange("p (t k) -> p t k", k=K)
            for t in range(T):
                nc.tensor.transpose(
                    sc_ps[:, t, :], t_sc1[:, t * P : (t + 1) * P], ident
                )
            # score = -0.5*v + log_mix (row-major packing)
            t_sc2 = persist.tile([P, T, K], f32)
            nc.vector.scalar_tensor_tensor(
                out=t_sc2, in0=sc_ps, scalar=-0.5,
                in1=t_lmx.rearrange("p (t k) -> p t k", k=K),
                op0=AL.mult, op1=AL.add,
            )
            t_nm = persist.tile([P, T], f32)
            nc.vector.tensor_reduce(
                out=t_nm, in_=t_sc2, axis=AX.X, op=AL.max, negate=True
            )
            nm_b = bass.AP(
                tensor=t_nm.tensor, offset=t_nm.offset,
                ap=[t_nm.ap[0], t_nm.ap[1], [0, K]],
            )
            t_es = persist.tile([P, T, K], f32)
            nc.vector.tensor_tensor(out=t_es, in0=t_sc2, in1=nm_b, op=AL.add)
            t_ex = persist.tile([P, T, K], f32)
            nc.scalar.activation(out=t_ex, in_=t_es, func=AF.Exp)
            t_S = persist.tile([P, T], f32)
            nc.vector.tensor_reduce(out=t_S, in_=t_ex, axis=AX.X, op=AL.add)
            t_lns = persist.tile([P, T], f32)
            nc.scalar.activation(out=t_lns, in_=t_S, func=AF.Ln)
            t_accv = persist.tile([P, T], f32)
            nc.vector.tensor_tensor(out=t_accv, in0=t_nm, in1=t_lns, op=AL.subtract)
            t_tot = persist.tile([P, 1], f32)
            nc.vector.tensor_reduce(out=t_tot, in_=t_accv, axis=AX.X, op=AL.add)
            nc.sync.dma_start(out=out, in_=t_tot)

    nc.compile()
    return nc


def get_nc():
    global _NC
    if _NC is None:
        _NC = _build()
    return _NC


def make_in_maps(target, s_mean, s_logstd, log_mix_coeffs):
    target = np.asarray(target, dtype=np.float32)
    s_mean = np.asarray(s_mean, dtype=np.float32)
    s_logstd = np.asarray(s_logstd, dtype=np.float32)
    lm = np.asarray(log_mix_coeffs, dtype=np.float32)
    in_maps = []
    for c in range(NCORES):
        rows = slice(c * R, (c + 1) * R)
        tgtT = np.ascontiguousarray(target[rows].T.astype(ml_dtypes.bfloat16))
        meanT = np.ascontiguousarray(
            s_mean[rows].reshape(R, K, D).transpose(2, 1, 0)
            .astype(ml_dtypes.bfloat16)
        )
        lstdT = np.ascontiguousarray(
            s_logstd[rows].reshape(R, K, D).transpose(2, 1, 0)
            .astype(ml_dtypes.float8_e4m3)
        )
        lmx = np.ascontiguousarray(
            lm[rows].reshape(T, P, K).transpose(1, 0, 2).reshape(P, T * K)
        )
        in_maps.append({
            "tgt": tgtT, "mean": meanT, "lstd": lstdT, "lmx": lmx,
            "ident": np.eye(K, dtype=np.float32),
        })
    return in_maps


def combine(results):
    total = sum(float(np.asarray(r["partial"], dtype=np.float64).sum()) for r in results)
    return np.float32(total / N)


def kernel(target, s_mean, s_logstd, log_mix_coeffs):
    from concourse.bass_utils import run_bass_kernel_spmd

    nc = get_nc()
    in_maps = make_in_maps(target, s_mean, s_logstd, log_mix_coeffs)
    res = run_bass_kernel_spmd(nc, in_maps, core_ids=list(range(NCORES)))
    return combine(res.results)
